# revision 1
# baseline (speedup 1.0000x reference)
"""Trainium2 Bass kernel for nn_DocREModel (doc-level relation extraction graph pooling).

Strategy (8 NeuronCores): each doc b (B=4) is split across 2 cores by attention
heads (6 heads each).  Every use of the attention tensor in the model is linear
in attention up to cheap scalar normalizations, so each core:
  - streams its [6,1024,1024] attention slice once from HBM,
  - accumulates the head-sum S[L,L] in SBUF (first head DMA'd straight into the
    accumulator, remaining heads added on the vector engine),
  - computes, via PE matmuls against host-built gather/mask matrices:
      GT     = S^T @ [onehotT|maskT]  (mention rows of S + span-row sums, both
                                       in contraction-major layout)
      v      = (uT*maskT)^T @ [seq|1]          (link-span numerator)
      mnum   = mrowsT^T @ [seq|1]              (mention-context numerator + row-sum)
      memb   = onehotT^T @ [seq|1]             (mention embeddings)
The host adds the two per-doc partials and applies the tiny normalizations
(head-count / span-length / row-sum divides, entity pooling, 4-way logsumexp)
while unsharding.
"""

import os
import sys

for _p in ("/opt/trn_rl_repo", "/root/.axon_site/_ro/trn_rl_repo"):
    if os.path.isdir(_p) and _p not in sys.path:
        sys.path.insert(0, _p)

import numpy as np

B, L, H, NH = 4, 1024, 768, 12
E, MPE, K = 32, 4, 16
EM = E * MPE              # 128 mentions per doc
TYPE_DIM = 20
OFFSET = 1
HPG = NH // 2             # heads per core (2 cores per doc)
RC = L // 128              # 8 chunks of 128 along L
HA = H + 2                # hidden + ones column (row-sum) + zero pad (fp32r needs even N)
RW = EM + K               # 144 real columns of the combined gather/mask matrix
RWP = 256                 # padded width so fp32r GT matmuls hit the 1cyc/row path

F32R_BIG = True           # float32r for the N>=256 contraction matmuls
F32R_GT = True            # float32r for the GT (S^T @ rmat) matmuls


def _build_nc(debug=False, f32r_big=F32R_BIG, f32r_gt=F32R_GT):
    import concourse.bass as bass
    import concourse.mybir as mybir
    import concourse.tile as tile
    from concourse import bacc

    f32 = mybir.dt.float32
    f32r = mybir.dt.float32r
    bf16 = mybir.dt.bfloat16
    ts, ds = bass.ts, bass.ds

    dm = f32r if (f32r_big or f32r_gt) else f32   # dtype for matmul operands

    def big(ap):
        return ap

    def gtc(ap):
        return ap

    nc = bacc.Bacc("TRN2", target_bir_lowering=False, debug=debug)

    att6 = nc.dram_tensor("att6", [HPG * L, L], bf16, kind="ExternalInput")
    seq_aug = nc.dram_tensor("seq_aug", [L, HA], dm, kind="ExternalInput")
    rmat = nc.dram_tensor("rmat", [L, RWP], dm, kind="ExternalInput")
    out_v = nc.dram_tensor("out_v", [K, HA], f32, kind="ExternalOutput")
    out_mnum = nc.dram_tensor("out_mnum", [EM, HA], f32, kind="ExternalOutput")
    out_memb = nc.dram_tensor("out_memb", [EM, HA], f32, kind="ExternalOutput")

    with tile.TileContext(nc) as tc:
        with (
            tc.tile_pool(name="const", bufs=1) as constp,
            tc.tile_pool(name="stream", bufs=12) as streamp,
            tc.tile_pool(name="accum", bufs=1) as accp,
            tc.tile_pool(name="stage", bufs=1) as stagep,
            tc.tile_pool(name="psall", bufs=8, space="PSUM") as psall,
        ):
            # ---- attention stream starts first (h=0 straight into S); consts
            #      interleave behind it so the HBM stream isn't delayed ----
            S_tiles = [accp.tile([128, L], dm, tag=f"S{rc}", name=f"S{rc}") for rc in range(RC)]
            gt_s = [accp.tile([128, RW], dm, tag=f"gt{ct}", name=f"gt{ct}") for ct in range(RC)]

            # consts loaded on the scalar queue right behind the first stream quad
            seq_s = constp.tile([128, RC, HA], dm, tag="seqs", name="seqs")
            rmat_s = constp.tile([128, RC, RWP], dm, tag="rmats", name="rmats")

            att6_r2 = att6[:].rearrange("(h rcq p) c -> h rcq p c", h=HPG, p=128)
            q0_tiles = []
            for h in range(HPG):
                t = streamp.tile([128, 4, L], bf16, tag="att", name="att")
                nc.sync.dma_start(out=t[:], in_=att6_r2[h, ds(0, 4)].rearrange("rcq p c -> p rcq c"))
                q0_tiles.append(t)
            for rc in range(RC):
                nc.scalar.dma_start(out=seq_s[:, rc, :], in_=seq_aug[ts(rc, 128), :])
                nc.scalar.dma_start(out=rmat_s[:, rc, :], in_=rmat[ts(rc, 128), :])

            # ---- mention embeddings memb = onehot^T @ [seq|1] (needs only consts) ----
            pmemb0 = psall.tile([EM, 512], f32, tag="ps", name="pmemb0")
            pmemb1 = psall.tile([EM, HA - 512], f32, tag="ps", name="pmemb1")
            for rc in range(RC):
                nc.tensor.matmul(pmemb0[:], big(rmat_s[:, rc, 0:EM]), big(seq_s[:, rc, 0:512]),
                                 start=(rc == 0), stop=(rc == RC - 1))
                nc.tensor.matmul(pmemb1[:], big(rmat_s[:, rc, 0:EM]), big(seq_s[:, rc, 512:HA]),
                                 start=(rc == 0), stop=(rc == RC - 1))
            memb_s = stagep.tile([EM, HA], f32, tag="memb", name="memb")
            nc.scalar.copy(out=memb_s[:, 0:512], in_=pmemb0[:])
            nc.scalar.copy(out=memb_s[:, 512:HA], in_=pmemb1[:])
            nc.sync.dma_start(out=out_memb[:], in_=memb_s[:])

            att6_r = att6[:].rearrange("(h rcq p) c -> h rcq p c", h=HPG, p=128)
            NQ = RC // 4  # two quads of four row-chunks
            groups = [list(range(0, 7)), list(range(7, 8))]  # GT groups: 7 + 1 chunks
            done_upto = 0
            for qq in range(NQ):
                if qq == 0:
                    tiles = q0_tiles
                else:
                    tiles = []
                    for h in range(HPG):
                        t = streamp.tile([128, 4, L], bf16, tag="att", name="att")
                        nc.sync.dma_start(out=t[:], in_=att6_r[h, ds(4 * qq, 4)].rearrange("rcq p c -> p rcq c"))
                        tiles.append(t)
                for j in range(4):
                    rc = 4 * qq + j
                    tp01 = streamp.tile([128, L], bf16, tag="tp", name="tp01", bufs=6)
                    tp23 = streamp.tile([128, L], bf16, tag="tp", name="tp23", bufs=6)
                    tp45 = streamp.tile([128, L], bf16, tag="tp", name="tp45", bufs=6)
                    nc.vector.tensor_add(tp01[:], tiles[0][:, j, :], tiles[1][:, j, :])
                    nc.vector.tensor_add(tp23[:], tiles[2][:, j, :], tiles[3][:, j, :])
                    nc.vector.tensor_add(tp45[:], tiles[4][:, j, :], tiles[5][:, j, :])
                    nc.vector.tensor_add(S_tiles[rc][:], tp01[:], tp23[:])
                    nc.vector.tensor_add(S_tiles[rc][:], S_tiles[rc][:], tp45[:])
                # GT group matmuls for every group fully covered by streamed chunks
                avail = 4 * qq + 4
                for gi, grp in enumerate(groups):
                    if grp[-1] < done_upto or grp[-1] >= avail:
                        continue
                    for ct in range(RC):
                        p = psall.tile([128, RWP], f32, tag="ps", name="gtq")
                        for j, rc in enumerate(grp):
                            nc.tensor.matmul(p[:], gtc(S_tiles[rc][:, ts(ct, 128)]), gtc(rmat_s[:, rc, :]),
                                             start=(j == 0), stop=(j == len(grp) - 1))
                        if gi == 0:
                            nc.scalar.copy(out=gt_s[ct][:], in_=p[:, 0:RW])
                        else:
                            nc.vector.tensor_add(gt_s[ct][:], gt_s[ct][:], p[:, 0:RW])
                    done_upto = grp[-1] + 1

            # ---- wvT = uT * maskT ----
            wv_s = [accp.tile([128, K], dm, tag=f"wv{ct}", name=f"wv{ct}") for ct in range(RC)]
            for ct in range(RC):
                nc.vector.tensor_mul(wv_s[ct][:], gt_s[ct][:, EM:RW], rmat_s[:, ct, EM:RW])

            # ---- contraction over positions: numerators for contexts + link reps ----
            pmnum0 = psall.tile([EM, 512], f32, tag="ps", name="pmnum0")
            pmnum1 = psall.tile([EM, HA - 512], f32, tag="ps", name="pmnum1")
            pv0 = psall.tile([K, 512], f32, tag="ps", name="pv0")
            pv1 = psall.tile([K, HA - 512], f32, tag="ps", name="pv1")
            for ct in range(RC):
                nc.tensor.matmul(pmnum0[:], big(gt_s[ct][:, 0:EM]), big(seq_s[:, ct, 0:512]),
                                 start=(ct == 0), stop=(ct == RC - 1))
                nc.tensor.matmul(pmnum1[:], big(gt_s[ct][:, 0:EM]), big(seq_s[:, ct, 512:HA]),
                                 start=(ct == 0), stop=(ct == RC - 1))
                nc.tensor.matmul(pv0[:], big(wv_s[ct][:]), big(seq_s[:, ct, 0:512]),
                                 start=(ct == 0), stop=(ct == RC - 1))
                nc.tensor.matmul(pv1[:], big(wv_s[ct][:]), big(seq_s[:, ct, 512:HA]),
                                 start=(ct == 0), stop=(ct == RC - 1))
            mnum_s = stagep.tile([EM, HA], f32, tag="mnum", name="mnum")
            nc.scalar.copy(out=mnum_s[:, 0:512], in_=pmnum0[:])
            nc.scalar.copy(out=mnum_s[:, 512:HA], in_=pmnum1[:])
            nc.sync.dma_start(out=out_mnum[:], in_=mnum_s[:])
            v_s = stagep.tile([K, HA], f32, tag="v", name="v")
            nc.scalar.copy(out=v_s[:, 0:512], in_=pv0[:])
            nc.scalar.copy(out=v_s[:, 512:HA], in_=pv1[:])
            nc.scalar.dma_start(out=out_v[:], in_=v_s[:])

    nc.compile()
    return nc


_NC_CACHE = {}


def _get_nc():
    if "nc" not in _NC_CACHE:
        _NC_CACHE["nc"] = _build_nc()
    return _NC_CACHE["nc"]


def _per_core_inputs(sequence_output, attention, mention_pos, link_start, link_len):
    """Returns (in_maps for 8 cores, per-doc span lengths)."""
    seq = np.ascontiguousarray(np.asarray(sequence_output, dtype=np.float32))
    import ml_dtypes
    att = np.asarray(attention)
    mpos = np.asarray(mention_pos).astype(np.int64)
    lstart = np.asarray(link_start).astype(np.int64)
    llen = np.asarray(link_len).astype(np.int64)

    in_maps = []
    lengths = []
    for b in range(B):
        pos = (mpos[b] + OFFSET).reshape(EM)
        onehotT = np.zeros((L, EM), np.float32)
        onehotT[pos, np.arange(EM)] = 1.0
        s = lstart[b] + OFFSET
        e = lstart[b] + llen[b] + 1 + OFFSET
        r = np.arange(L)
        maskT = ((r[:, None] >= s[None, :]) & (r[:, None] < e[None, :])).astype(np.float32)
        rmat = np.ascontiguousarray(np.concatenate(
            [onehotT, maskT, np.zeros((L, RWP - RW), np.float32)], axis=1))
        seq_aug = np.ascontiguousarray(
            np.concatenate([seq[b], np.ones((L, 1), np.float32), np.zeros((L, 1), np.float32)], axis=1))
        lengths.append((e - s).astype(np.float32))
        for g in range(2):
            att6 = np.ascontiguousarray(
                att[b, g * HPG:(g + 1) * HPG].reshape(HPG * L, L).astype(ml_dtypes.bfloat16))
            in_maps.append({"att6": att6, "seq_aug": seq_aug, "rmat": rmat})
    return in_maps, lengths


def _combine(outs, lengths, type_table):
    ttab = np.asarray(type_table, dtype=np.float32)
    type_ids = np.concatenate(
        [np.zeros(E, np.int64), np.ones(EM, np.int64), np.full(K, 2, np.int64)])
    nodes_type = ttab[type_ids]  # [E+EM+K, TYPE_DIM]

    out = np.zeros((B, E + EM + K + E + EM, H + TYPE_DIM), np.float32)
    for b in range(B):
        o0, o1 = outs[2 * b], outs[2 * b + 1]
        v = o0["out_v"] + o1["out_v"]
        mnum = o0["out_mnum"] + o1["out_mnum"]
        memb = o0["out_memb"][:, :H]
        length = lengths[b]

        link_rep = v[:, :H] / (NH * length[:, None])
        m_ctx = mnum[:, :H] / (mnum[:, H:H + 1] + NH * 1e-5)
        enum = mnum.reshape(E, MPE, HA).sum(axis=1)
        e_ctx = enum[:, :H] / (enum[:, H:H + 1] + NH * MPE * 1e-5)

        mg = memb.reshape(E, MPE, H)
        mmax = mg.max(axis=1)
        eemb = np.log(np.exp(mg - mmax[:, None, :]).sum(axis=1)) + mmax

        nodes_raw = np.concatenate([eemb, memb, link_rep], axis=0)      # [176,H]
        nodes = np.concatenate([nodes_raw, nodes_type], axis=1)         # [176,H+20]
        ctx = np.concatenate([e_ctx, m_ctx], axis=0)                    # [160,H]
        ctx = np.concatenate([ctx, np.zeros((E + EM, TYPE_DIM), np.float32)], axis=1)
        out[b] = np.concatenate([nodes, ctx], axis=0)
    return out


def kernel(**inputs):
    from concourse.bass_utils import run_bass_kernel_spmd

    in_maps, lengths = _per_core_inputs(
        inputs["sequence_output"], inputs["attention"],
        inputs["mention_pos"], inputs["link_start"], inputs["link_len"])
    nc = _get_nc()
    res = run_bass_kernel_spmd(nc, in_maps, core_ids=list(range(8)))
    return _combine(res.results, lengths, inputs["type_table"])



# revision 3
# speedup vs baseline: 2.3018x; 2.3018x over previous
"""Trainium2 Bass kernel for nn_DocREModel (doc-level relation extraction graph pooling).

Key observation: attention only enters the model through (a) rows at the 128
mention positions and (b) rows inside the 16 link spans -- ~350 of 1024 rows per
doc -- and every use is linear in the head-summed attention.  So the host
gathers exactly those rows (pure data movement, like the one-hot matrices it
already built) and each core streams ~6.7 MB instead of ~16 MB.

Sharding (8 cores): doc b -> core pair (2b, 2b+1), each handling 6 of the 12
attention heads.  Per core:
  - mention block arrives host-TRANSPOSED as [c-part, (head, ct, mention)]; five
    bf16 DVE adds give Msum^T = the gathered mention-attention already in the
    layout the context matmul needs (no PE transposes, no PSUM cycling),
  - span blocks arrive row-major [slot, (head, c)]; five bf16 adds per block
    give the head-sum HS, then matmuls vs the 0/1 span-membership matrix
    produce the span-row sums u^T[c,k] (PSUM-accumulated),
  - wv = u^T * mask^T (one DVE mul), then two PSUM-accumulated matmul chains
    against seq_aug=[seq | 1] produce the mention-context numerators (+row sums)
    and link-span numerators.
The host adds the two per-doc partials, applies the tiny normalizations
(row-sum / span-length / head-count divides), gathers mention embeddings
directly from sequence_output (exact), and does the 4-way logsumexp pooling.
"""

import os
import sys

for _p in ("/opt/trn_rl_repo", "/root/.axon_site/_ro/trn_rl_repo"):
    if os.path.isdir(_p) and _p not in sys.path:
        sys.path.insert(0, _p)

import numpy as np

B, L, H, NH = 4, 1024, 768, 12
E, MPE, K = 32, 4, 16
EM = E * MPE              # 128 mentions per doc
TYPE_DIM = 20
OFFSET = 1
HPG = NH // 2             # heads per core (2 cores per doc)
RC = L // 128             # 8 chunks of 128 along c
HA = H + 4                # hidden + ones column + pad to 772
N0 = 512                  # first PSUM bank width for the HA-dim matmuls
N1 = HA - N0              # 260


def _build_nc(nsp, debug=False):
    """nsp = number of 128-row span blocks (global max over docs)."""
    import concourse.bass as bass
    import concourse.mybir as mybir
    import concourse.tile as tile
    from concourse import bacc

    f32 = mybir.dt.float32
    bf16 = mybir.dt.bfloat16
    ts, ds = bass.ts, bass.ds

    nc = bacc.Bacc("TRN2", target_bir_lowering=False, debug=debug)

    # host-prepped inputs (all partition-major)
    gsp = nc.dram_tensor("gsp", [nsp * 128, HPG * L], bf16, kind="ExternalInput")
    gmt = nc.dram_tensor("gmt", [128, HPG * RC * EM], bf16, kind="ExternalInput")
    seqp = nc.dram_tensor("seqp", [128, RC * HA], bf16, kind="ExternalInput")
    wsp = nc.dram_tensor("wsp", [128, nsp * K], bf16, kind="ExternalInput")
    maskcp = nc.dram_tensor("maskcp", [128, RC * K], bf16, kind="ExternalInput")
    out_mnum = nc.dram_tensor("out_mnum", [EM, HA], f32, kind="ExternalOutput")
    out_v = nc.dram_tensor("out_v", [K, HA], f32, kind="ExternalOutput")

    MB = RC * EM              # mention block cols per head (1024)
    HC = RC // 2              # ct chunks per half

    with tile.TileContext(nc) as tc:
        with (
            tc.tile_pool(name="const", bufs=1) as constp,
            tc.tile_pool(name="stream", bufs=max(3, nsp + 1)) as streamp,
            tc.tile_pool(name="tmp", bufs=2) as tmpp,
            tc.tile_pool(name="acc", bufs=1) as accp,
            tc.tile_pool(name="stage", bufs=1) as stagep,
            tc.tile_pool(name="psall", bufs=5, space="PSUM") as psall,
        ):
            # ---- DMA: seq + small consts on scalar queue; span blocks then the
            #      mention block on the sync queue (mention branch runs last) ----
            seq_s = constp.tile([128, RC * HA], bf16, name="seqs")
            nc.scalar.dma_start(out=seq_s[:], in_=seqp[:])
            wsp_s = constp.tile([128, nsp * K], bf16, name="wsps")
            nc.scalar.dma_start(out=wsp_s[:], in_=wsp[:])
            maskc_s = constp.tile([128, RC * K], bf16, name="maskcs")
            nc.scalar.dma_start(out=maskc_s[:], in_=maskcp[:])

            gsp_t = []
            for sc in range(nsp):
                t = streamp.tile([128, HPG * L], bf16, tag="gsp", name=f"gsp{sc}")
                nc.sync.dma_start(out=t[:], in_=gsp[ts(sc, 128), :])
                gsp_t.append(t)
            gmt_t = streamp.tile([128, HPG * MB], bf16, tag="gmt", name="gmt")
            nc.sync.dma_start(out=gmt_t[:], in_=gmt[:])

            # ---- span branch: head-sum per block, then u^T[c,k] via W matmuls ----
            hss = [accp.tile([128, L], bf16, tag=f"hss{sc}", name=f"hss{sc}") for sc in range(nsp)]
            for sc in range(nsp):
                t = gsp_t[sc]
                a01 = tmpp.tile([128, L], bf16, tag="sa", name="a01")
                a23 = tmpp.tile([128, L], bf16, tag="sb", name="a23")
                a45 = tmpp.tile([128, L], bf16, tag="sc", name="a45")
                nc.vector.tensor_add(a01[:], t[:, ts(0, L)], t[:, ts(1, L)])
                nc.vector.tensor_add(a23[:], t[:, ts(2, L)], t[:, ts(3, L)])
                nc.vector.tensor_add(a45[:], t[:, ts(4, L)], t[:, ts(5, L)])
                nc.vector.tensor_add(a01[:], a01[:], a23[:])
                nc.vector.tensor_add(hss[sc][:], a01[:], a45[:])

            pgs = psall.tile([128, RC * K], f32, tag="ps", name="pgs")
            for ct in range(RC):
                for sc in range(nsp):
                    nc.tensor.matmul(pgs[:, ts(ct, K)], hss[sc][:, ts(ct, 128)],
                                     wsp_s[:, ts(sc, K)],
                                     start=(sc == 0), stop=(sc == nsp - 1))
            wv = accp.tile([128, RC * K], bf16, tag="wv", name="wv")
            nc.vector.tensor_mul(wv[:], pgs[:], maskc_s[:])

            pv0 = psall.tile([K, N0], f32, tag="ps", name="pv0")
            pv1 = psall.tile([K, N1], f32, tag="ps", name="pv1")
            for ct in range(RC):
                nc.tensor.matmul(pv0[:], wv[:, ts(ct, K)], seq_s[:, ds(ct * HA, N0)],
                                 start=(ct == 0), stop=(ct == RC - 1))
                nc.tensor.matmul(pv1[:], wv[:, ts(ct, K)], seq_s[:, ds(ct * HA + N0, N1)],
                                 start=(ct == 0), stop=(ct == RC - 1))
            v_s = stagep.tile([K, HA], f32, tag="vs", name="vs")
            nc.scalar.copy(out=v_s[:, 0:N0], in_=pv0[:])
            nc.scalar.copy(out=v_s[:, N0:HA], in_=pv1[:])
            nc.scalar.dma_start(out=out_v[:], in_=v_s[:])

            # ---- mention branch: head-sum (arrives c-part transposed) + mnum chain;
            #      adds split in ct-halves so the matmul chain starts at half-time ----
            hsm = accp.tile([128, MB], bf16, tag="hsm", name="hsm")
            pmn0 = psall.tile([EM, N0], f32, tag="ps", name="pmn0")
            pmn1 = psall.tile([EM, N1], f32, tag="ps", name="pmn1")
            HB = HC * EM          # cols per half (512)
            for half in range(2):
                m01 = tmpp.tile([128, HB], bf16, tag="ma", name="m01")
                m23 = tmpp.tile([128, HB], bf16, tag="mb", name="m23")
                m45 = tmpp.tile([128, HB], bf16, tag="mc", name="m45")
                off = half * HB
                nc.vector.tensor_add(m01[:], gmt_t[:, ds(0 * MB + off, HB)],
                                     gmt_t[:, ds(1 * MB + off, HB)])
                nc.vector.tensor_add(m23[:], gmt_t[:, ds(2 * MB + off, HB)],
                                     gmt_t[:, ds(3 * MB + off, HB)])
                nc.vector.tensor_add(m45[:], gmt_t[:, ds(4 * MB + off, HB)],
                                     gmt_t[:, ds(5 * MB + off, HB)])
                nc.vector.tensor_add(m01[:], m01[:], m23[:])
                nc.vector.tensor_add(hsm[:, ds(off, HB)], m01[:], m45[:])
                for j in range(HC):
                    ct = half * HC + j
                    nc.tensor.matmul(pmn0[:], hsm[:, ts(ct, EM)], seq_s[:, ds(ct * HA, N0)],
                                     start=(ct == 0), stop=(ct == RC - 1))
                    nc.tensor.matmul(pmn1[:], hsm[:, ts(ct, EM)], seq_s[:, ds(ct * HA + N0, N1)],
                                     start=(ct == 0), stop=(ct == RC - 1))
            mnum_s = stagep.tile([EM, HA], f32, tag="mns", name="mns")
            nc.scalar.copy(out=mnum_s[:, 0:N0], in_=pmn0[:])
            nc.scalar.copy(out=mnum_s[:, N0:HA], in_=pmn1[:])
            nc.sync.dma_start(out=out_mnum[:], in_=mnum_s[:])

    nc.compile()
    return nc


_NC_CACHE = {}


def _get_nc(nsp=2):
    if nsp not in _NC_CACHE:
        _NC_CACHE[nsp] = _build_nc(nsp)
    return _NC_CACHE[nsp]


def _per_core_inputs(sequence_output, attention, mention_pos, link_start, link_len):
    """Returns (in_maps for 8 cores, per-doc span lengths, nsp)."""
    import ml_dtypes
    bf16 = ml_dtypes.bfloat16
    seq = np.asarray(sequence_output, dtype=np.float32)
    att = np.asarray(attention)
    mpos = np.asarray(mention_pos).astype(np.int64)
    lstart = np.asarray(link_start).astype(np.int64)
    llen = np.asarray(link_len).astype(np.int64)

    doc = []
    max_u = 1
    for b in range(B):
        pos = (mpos[b] + OFFSET).reshape(EM)
        s = lstart[b] + OFFSET
        e = lstart[b] + llen[b] + 1 + OFFSET
        srows = np.unique(np.concatenate([np.arange(si, ei) for si, ei in zip(s, e)]))
        max_u = max(max_u, len(srows))
        doc.append((pos, s, e, srows))
    nsp = (max_u + 127) // 128

    in_maps = []
    lengths = []
    for b in range(B):
        pos, s, e, srows = doc[b]
        nsr = len(srows)
        srows_p = np.zeros(nsp * 128, np.int64)
        srows_p[:nsr] = srows
        # span membership [slot, k] and column mask [c, k], partition-major
        wspm = np.zeros((nsp * 128, K), np.float32)
        wspm[:nsr] = ((srows[:, None] >= s[None, :]) & (srows[:, None] < e[None, :]))
        wsp_p = np.ascontiguousarray(
            wspm.reshape(nsp, 128, K).transpose(1, 0, 2).reshape(128, nsp * K)).astype(bf16)
        r = np.arange(L)
        maskc = ((r[:, None] >= s[None, :]) & (r[:, None] < e[None, :])).astype(np.float32)
        maskc_p = np.ascontiguousarray(
            maskc.reshape(RC, 128, K).transpose(1, 0, 2).reshape(128, RC * K)).astype(bf16)
        seq_aug = np.concatenate(
            [seq[b], np.ones((L, 1), np.float32), np.zeros((L, HA - H - 1), np.float32)], axis=1)
        seqp = np.ascontiguousarray(
            seq_aug.reshape(RC, 128, HA).transpose(1, 0, 2).reshape(128, RC * HA)).astype(bf16)
        lengths.append((e - s).astype(np.float32))
        for g in range(2):
            hsl = slice(g * HPG, (g + 1) * HPG)
            # span rows, row-major: [sc*128+q, h*L+c]
            gspr = att[b, hsl][:, srows_p, :]                      # [HPG, nsp*128, L]
            gsp = np.ascontiguousarray(
                gspr.transpose(1, 0, 2).reshape(nsp * 128, HPG * L)).astype(bf16)
            # mention rows, transposed: [p, h*RC*EM + ct*EM + m]
            gmtr = att[b, hsl][:, pos, :]                          # [HPG, EM, L]
            gmtx = np.ascontiguousarray(
                gmtr.reshape(HPG, EM, RC, 128).transpose(3, 0, 2, 1).reshape(128, HPG * RC * EM)
            ).astype(bf16)
            in_maps.append({"gsp": gsp, "gmt": gmtx, "seqp": seqp,
                            "wsp": wsp_p, "maskcp": maskc_p})
    return in_maps, lengths, nsp


def _combine(outs, lengths, sequence_output, type_table, mention_pos):
    seq = np.asarray(sequence_output, dtype=np.float32)
    mpos = np.asarray(mention_pos).astype(np.int64)
    ttab = np.asarray(type_table, dtype=np.float32)
    type_ids = np.concatenate(
        [np.zeros(E, np.int64), np.ones(EM, np.int64), np.full(K, 2, np.int64)])
    nodes_type = ttab[type_ids]  # [E+EM+K, TYPE_DIM]

    out = np.zeros((B, E + EM + K + E + EM, H + TYPE_DIM), np.float32)
    for b in range(B):
        o0, o1 = outs[2 * b], outs[2 * b + 1]
        v = o0["out_v"] + o1["out_v"]
        mnum = o0["out_mnum"] + o1["out_mnum"]
        length = lengths[b]

        link_rep = v[:, :H] / (NH * length[:, None])
        m_ctx = mnum[:, :H] / (mnum[:, H:H + 1] + NH * 1e-5)
        enum = mnum.reshape(E, MPE, HA).sum(axis=1)
        e_ctx = enum[:, :H] / (enum[:, H:H + 1] + NH * MPE * 1e-5)

        pos = (mpos[b] + OFFSET).reshape(EM)
        memb = seq[b, pos]                                          # exact gather
        mg = memb.reshape(E, MPE, H)
        mmax = mg.max(axis=1)
        eemb = np.log(np.exp(mg - mmax[:, None, :]).sum(axis=1)) + mmax

        nodes_raw = np.concatenate([eemb, memb, link_rep], axis=0)  # [176,H]
        nodes = np.concatenate([nodes_raw, nodes_type], axis=1)     # [176,H+20]
        ctx = np.concatenate([e_ctx, m_ctx], axis=0)                # [160,H]
        ctx = np.concatenate([ctx, np.zeros((E + EM, TYPE_DIM), np.float32)], axis=1)
        out[b] = np.concatenate([nodes, ctx], axis=0)
    return out


def kernel(**inputs):
    from concourse.bass_utils import run_bass_kernel_spmd

    in_maps, lengths, nsp = _per_core_inputs(
        inputs["sequence_output"], inputs["attention"],
        inputs["mention_pos"], inputs["link_start"], inputs["link_len"])
    nc = _get_nc(nsp)
    res = run_bass_kernel_spmd(nc, in_maps, core_ids=list(range(8)))
    return _combine(res.results, lengths, inputs["sequence_output"],
                    inputs["type_table"], inputs["mention_pos"])
